# revision 54
# baseline (speedup 1.0000x reference)
"""Biaffine attention kernel for Trainium2, data-parallel over 8 NeuronCores.

Math (per batch b, X = x[b] (128, L), Y = y[b]):
    xp = Wf X + bf 1^T,  yp = Wa Y + ba 1^T
    scores = xp yp^T
           = Wf (X Y^T) Wa^T + (Wf X 1) ba^T + bf (Wa Y 1)^T + L bf ba^T
    attn = softmax(scores, -1) / sqrt(L)
    out  = attn (xp + yp) = (attn Wf) X + (attn Wa) Y + (attn (bf+ba)) 1^T

Distribution: batch dim (32) sharded 4-per-core across 8 cores; weights
replicated. No collectives.

Implementation notes (per core):
  - x/y are cast to fp16 on the host; all HBM traffic is fp16 (in 4 MiB +
    out 2 MiB per batch vs 12 MiB for fp32), out is upcast host-side.
  - all small constants (Wf/Wa natural + transposed fp16, identity, bias
    rows, bf+ba column, score-bias rank-1 rows) are precomputed on the host
    and shipped as two packed tensors (one [128,641] + one [1,1280] DMA),
    so the kernel needs no on-chip identity build or weight transposes and
    the first gen matmul only waits on the first x/y piece.
  - a block of warm-up matmuls on a zero tile runs while the first inputs
    stream in, so the PE HAM clock-gate reaches 8/8 before the first real
    matmul (otherwise the first ~15 us run at half clock); zero-valued
    filler matmuls also pad the load-paced batch-0 ramp.
  - "gen": xpT chunks = matmul(lhsT=X_c, rhs=WfT) produce the transposed
    projections directly from the natural-layout input; PSUM groups are
    1024 cols (2 banks) so each ACT/DVE evacuation moves twice as much per
    instruction (the fixed cost per evac op is ~40% of a 512-col evac).
  - the per-batch pipeline is one fused phase: gen(b+1) groups, scores(b+1)
    chunk matmuls (one gen-pair behind the groups producing their inputs),
    and tail(b) out chunks interleave on the PE so ACT/DVE always have a
    window of evacuation work while the PE fills the next PSUM tile —
    no phase is evacuation-gated and the PE never idles long enough for
    the HAM clock-gate to re-throttle mid-kernel.
  - scores accumulate over the 64 transposed chunks into one PSUM bank;
    the three bias rank-1 terms are added exactly via two k=1 matmuls
    (u ba^T and bf (Wa sy + L ba)^T) from host-precomputed rows. The
    attn-derived smalls (attnT/AfT/AaT/attnc) live in a second psc bank
    that alternates with the scores bank, so scores(b+1) never waits on
    tail(b)'s reads.
  - out = AfT.T @ X + AaT.T @ Y consumed from the still-resident raw fp16
    inputs, in 512-col PSUM chunks, with the attn(bf+ba) column folded in
    as the PSUM-evacuation bias. The lone final tail alternates its chunk
    banks across the pout AND (by then idle) pgen pools and its stores
    across both HWDGE rings, with 4 staging buffers.
  - stores: SWDGE ring for early batches (gpsimd is otherwise idle), SP
    ring for the last two (loads done; HWDGE receipt keeps the tail short).
  - the second-to-last tail's last HOLD chunks are deferred past the last
    softmax so the PE stays busy through its serial chain.
"""

import numpy as np

P = 128
L = 8192
B = 32
NCORES = 8
BPC = B // NCORES  # batches per core
SQRT_L = float(np.sqrt(float(L)))

GEN_GRP = 8  # gen chunks per PSUM evacuation (8*128 fp32 = 2 banks)
OUT_TILE = 4096  # out staging tile (fp16) per DMA store
OUT_CHUNK = 512  # out matmul PSUM chunk (1 bank)
IN_PIECE = 4096  # input DMA piece (cols; 2 MiB transfers run nearer peak BW)
NWARM = 50  # PE warm-up matmuls (N=128 each, ~5 us at the cold clock)
HOLD = 5  # tail chunks deferred past the last softmax

# wpack column layout (fp16, [128, 641])
WP_WFT = 0  # Wf^T  (f x o)
WP_WAT = 128  # Wa^T  (a x o)
WP_WF = 256  # Wf    (o x f)
WP_WA = 384  # Wa    (o x a)
WP_ID = 512  # identity
WP_C = 640  # bf+ba column
WP_COLS = 641
# rpack column layout (fp16, [1, 256 + 2*BPC*128])
RP_BF = 0
RP_BA = 128
RP_U = 256
RP_W = 256 + BPC * P
RP_COLS = 256 + 2 * BPC * P


def _patch_tail_drain(tile, mybir, ScopedClock):
    """This container's walrus rejects >1 sync wait on the kernel-tail Drain
    (setupSyncWait: 'Too many sync wait commands'). Spread the tail-drain
    waits across a chain of drains, one wait each."""
    if getattr(tile.TileContext, "_drain_split_patched", False):
        return

    def _split_drain_and_barrier(self, tick_clock, wait_clock):
        nc = self.nc
        drain_inst = nc.sync.drain()
        wait_clock.add_sem_waits(
            drain_inst.ins, ScopedClock({None: tick_clock.global_clock})
        )
        si = drain_inst.ins.sync_info
        if si is not None and si.on_wait is not None and len(si.on_wait) > 1:
            waits = list(si.on_wait)
            si.on_wait = waits[:1]
            # spread the remaining waits across all engines as parallel
            # single-wait NOP chains (walrus allows only one wait per
            # instruction; a serial chain on SP costs ~4us of kernel tail)
            engines = [nc.tensor, nc.vector, nc.scalar, nc.gpsimd, nc.sync]
            for i, w in enumerate(waits[1:]):
                extra = engines[i % len(engines)].nop(nofuse=True)
                esi = extra.ins.sync_info
                if esi is None:
                    extra.ins.sync_info = mybir.SyncInfo(on_wait=[w], on_update=[])
                else:
                    ow = list(esi.on_wait) if esi.on_wait else []
                    ow.append(w)
                    esi.on_wait = ow
        nc.all_engine_barrier(sem_only=True)
        assert self.sems is not None
        popped = nc._tile_sem_poison_stack.pop()
        assert popped is self._sem_poison
        nc.clear_and_free_semaphores(list(self.sems.allocated().values()))
        nc.all_engine_barrier(sem_only=True)

    tile.TileContext._drain_and_barrier = _split_drain_and_barrier
    tile.TileContext._drain_split_patched = True


def _split_excess_waits(nc, mybir, max_waits=1):
    """Walrus in this container rejects instructions carrying more than a
    couple of sync waits ('Too many sync wait commands'). Hoist excess waits
    onto dedicated same-engine NoOps inserted just before the instruction."""
    ctr = 0
    for blk in nc.m.functions[0].blocks:
        new_insts = []
        for inst in blk.instructions:
            si = inst.sync_info
            if si is not None and si.on_wait and len(si.on_wait) > max_waits:
                waits = list(si.on_wait)
                excess, keep = waits[:-max_waits], waits[-max_waits:]
                si.on_wait = keep
                for i in range(0, len(excess), max_waits):
                    ctr += 1
                    nop = mybir.InstNoOp(
                        name=f"I-waitsplit-{ctr}",
                        sync_info=mybir.SyncInfo(
                            on_wait=excess[i : i + max_waits], on_update=[]
                        ),
                        bass_nofuse=True,
                        engine=inst.engine,
                    )
                    nc.register_instruction(nop)
                    new_insts.append(nop)
            new_insts.append(inst)
        blk.instructions = new_insts


def build_nc(bpc=BPC, seq=L):
    import concourse.bass as bass
    import concourse.mybir as mybir
    import concourse.tile as tile
    from concourse.vector_clock import ScopedClock

    _patch_tail_drain(tile, mybir, ScopedClock)

    f32 = mybir.dt.float32
    f16 = mybir.dt.float16
    AF = mybir.ActivationFunctionType
    ALU = mybir.AluOpType
    AX = mybir.AxisListType

    sqrt_l = float(np.sqrt(float(seq)))
    ntr = seq // P  # number of 128-col chunks
    ngrp = ntr // GEN_GRP
    npc = seq // IN_PIECE  # input dma pieces

    nc = bass.Bass("TRN2", target_bir_lowering=False, debug=False)
    x_d = nc.dram_tensor("x", [bpc, P, seq], f16, kind="ExternalInput").ap()
    y_d = nc.dram_tensor("y", [bpc, P, seq], f16, kind="ExternalInput").ap()
    wpack_d = nc.dram_tensor("wpack", [P, WP_COLS], f16, kind="ExternalInput").ap()
    rpack_d = nc.dram_tensor("rpack", [1, RP_COLS], f16, kind="ExternalInput").ap()
    out_d = nc.dram_tensor("out", [bpc, P, seq], f16, kind="ExternalOutput").ap()

    def load_inputs(b, x_t, y_t):
        # Column-split dual-ring loads: the front half (consumed first by
        # the gen pairs, x/y interleaved in arrival order) rides the SP
        # ring; the back half streams in parallel on the ACT ring, so it
        # arrives concurrently instead of queuing behind the front.
        if b == 0:
            front = [0, 1024, 2048, 4096]
        elif b == 1:
            # gen(1) inside fused(1,0) tracks these arrivals closely:
            # finer pieces unblock its first pairs sooner
            front = [0, 2048, 4096]
        else:
            front = [0, 4096]
        back = [4096, 6144, seq] if b <= 1 else [4096, seq]
        for eng, edges in ((nc.sync, front), (nc.scalar, back)):
            for e0, e1 in zip(edges[:-1], edges[1:]):
                ps_ = slice(e0, e1)
                eng.dma_start(x_t[:, ps_], x_d[b, :, ps_])
                eng.dma_start(y_t[:, ps_], y_d[b, :, ps_])

    with tile.TileContext(nc) as tc:
        with (
            tc.tile_pool(name="consts", bufs=1) as consts,
            tc.tile_pool(name="xin", bufs=3) as xin_pool,
            tc.tile_pool(name="acts", bufs=2) as acts_pool,
            tc.tile_pool(name="sm", bufs=2) as sm_pool,
            tc.tile_pool(name="outs", bufs=2) as out_pool,
            tc.tile_pool(name="pgen", bufs=2, space="PSUM") as psum_gen,
            tc.tile_pool(name="pout", bufs=2, space="PSUM") as psum_out,
            tc.tile_pool(name="psc", bufs=2, space="PSUM") as psum_sc,
        ):
            # ---- PE warm-up: dummy matmuls on a zero tile while the first
            # inputs stream in. Keeps the PE HAM busy-window fed so the
            # clock gate opens (4/8 -> 8/8) before the first real matmul.
            # The warm bank comes from the psc pool; its slot is free again
            # immediately (nothing reads it). ----
            warm_ps = psum_sc.tile([P, 4 * P], f32, tag="ps", name="warm_ps")
            wtile = consts.tile([P, P], f16)
            nc.vector.memset(wtile, 0.0)
            for _ in range(NWARM):
                nc.tensor.matmul(warm_ps[:, 0:P], wtile, wtile, start=True, stop=True)

            # ---- constants: one packed DMA each on the ACT ring ----
            wpk = consts.tile([P, WP_COLS], f16)
            nc.scalar.dma_start(wpk, wpack_d)
            rpk = consts.tile([1, RP_COLS], f16)
            nc.scalar.dma_start(rpk, rpack_d)
            wfT16 = wpk[:, WP_WFT : WP_WFT + P]
            waT16 = wpk[:, WP_WAT : WP_WAT + P]
            wf16 = wpk[:, WP_WF : WP_WF + P]
            wa16 = wpk[:, WP_WA : WP_WA + P]
            ids16 = wpk[:, WP_ID : WP_ID + P]
            c_col16 = wpk[:, WP_C : WP_C + 1]
            bf_row16 = rpk[:, RP_BF : RP_BF + P]
            ba_row16 = rpk[:, RP_BA : RP_BA + P]

            xy_tiles = {}

            def issue_load(b):
                if b >= bpc:
                    return
                x_t = xin_pool.tile([P, seq], f16, tag="x_t", name="x_t")
                y_t = xin_pool.tile([P, seq], f16, tag="y_t", name="y_t")
                load_inputs(b, x_t, y_t)
                xy_tiles[b] = (x_t, y_t)

            issue_load(0)
            for b0 in range(1, min(3, bpc)):
                issue_load(b0)

            def gen_group(src, wT, dst, g, eng):
                # one gen group: GEN_GRP transposed-projection matmuls into a
                # 2-bank PSUM tile, then a single 1024-col evacuation
                pg = psum_gen.tile([P, GEN_GRP * P], f32, tag="pg", name="pg")
                for t in range(GEN_GRP):
                    c = g * GEN_GRP + t
                    cs = slice(c * P, (c + 1) * P)
                    ts_ = slice(t * P, (t + 1) * P)
                    nc.tensor.matmul(
                        pg[:, ts_], src[:, cs], wT, start=True, stop=True
                    )
                gs = slice(g * GEN_GRP * P, (g + 1) * GEN_GRP * P)
                if eng is nc.scalar:
                    nc.scalar.activation(out=dst[:, gs], in_=pg, func=AF.Identity)
                else:
                    nc.vector.tensor_copy(out=dst[:, gs], in_=pg)

            class ScoreAcc:
                """scores(b) = sum_c xpT_c^T ypT_c + rank-1 bias terms,
                accumulated into one PSUM bank. The chunk matmuls are issued
                incrementally (one gen-pair behind the gen groups producing
                xpT/ypT) so they fill PE time while ACT/DVE drain PSUM."""

                def __init__(self, b, xpT, ypT):
                    self.b = b
                    self.xpT, self.ypT = xpT, ypT
                    self.pbank = psum_sc.tile(
                        [P, 4 * P], f32, tag="ps", name="pbank"
                    )
                    self.done = 0

                def run_to(self, c_hi):
                    ps = self.pbank[:, 0:P]
                    for c in range(self.done, c_hi):
                        cs = slice(c * P, (c + 1) * P)
                        nc.tensor.matmul(
                            ps, self.xpT[:, cs], self.ypT[:, cs],
                            start=(c == 0), stop=False,
                        )
                    self.done = max(self.done, c_hi)

                def finish(self):
                    self.run_to(ntr)
                    ps = self.pbank[:, 0:P]
                    us = slice(RP_U + self.b * P, RP_U + (self.b + 1) * P)
                    ws = slice(RP_W + self.b * P, RP_W + (self.b + 1) * P)
                    nc.tensor.matmul(
                        ps, rpk[:, us], ba_row16, start=False, stop=False
                    )
                    nc.tensor.matmul(
                        ps, bf_row16, rpk[:, ws], start=False, stop=True
                    )
                    return self.pbank

            def do_gen_scores(b):
                # batch-0 bootstrap: gen pairs paced by the input stream,
                # with scores chunks trailing one pair behind. Zero-valued
                # filler matmuls (accumulating 0 into unused scores-bank
                # columns) keep the PE HAM busy-fraction up while pairs wait
                # on the DMA stream, so the clock gate stays at 8/8.
                x_t, y_t = xy_tiles[b]
                xpT = acts_pool.tile([P, seq], f16, tag="xpT", name="xpT")
                ypT = acts_pool.tile([P, seq], f16, tag="ypT", name="ypT")
                acc = ScoreAcc(b, xpT, ypT)
                for g in range(ngrp):
                    gen_group(x_t, wfT16, xpT, g, nc.scalar)
                    gen_group(y_t, waT16, ypT, g, nc.vector)
                    acc.run_to(g * GEN_GRP)
                    for _ in range(8):
                        nc.tensor.matmul(
                            acc.pbank[:, 3 * P : 4 * P], wtile, wtile,
                            start=False, stop=False,
                        )
                return xpT, ypT, acc.finish()

            def do_softmax_a(b, pbank):
                # softmax part 1: max-reduce + exp (DVE + ACT)
                ps = pbank[:, 0:P]
                negmx = sm_pool.tile([P, 1], f32, tag="negmx", name="negmx")
                nc.vector.tensor_reduce(
                    out=negmx, in_=ps, axis=AX.X, op=ALU.max, negate=True
                )
                e = sm_pool.tile([P, P], f32, tag="e", name="e")
                se = sm_pool.tile([P, 1], f32, tag="se", name="se")
                nc.scalar.activation(
                    out=e, in_=ps, func=AF.Exp, bias=negmx, scale=1.0, accum_out=se
                )
                return e, se

            def do_softmax_b(b, e, se):
                # softmax part 2: normalize (DVE only)
                sse = sm_pool.tile([P, 1], f32, tag="sse", name="sse")
                nc.vector.tensor_scalar_mul(sse, se, sqrt_l)
                rcp = sm_pool.tile([P, 1], f32, tag="rcp", name="rcp")
                nc.vector.reciprocal(rcp, sse)
                attn = sm_pool.tile([P, P], f16, tag="attn", name="attn")
                nc.vector.tensor_scalar_mul(attn, e, rcp)
                return attn

            def do_softmax(b, pbank):
                e, se = do_softmax_a(b, pbank)
                return do_softmax_b(b, e, se)

            def do_smalls_a(b, attn):
                # attn transpose, in its own psc bank (so the scores bank
                # frees as soon as exp has read it)
                msm = psum_sc.tile([P, 4 * P], f32, tag="ps", name="msm")
                pat = msm[:, 0 : P // 2].bitcast(f16)
                nc.tensor.transpose(pat, attn, ids16)
                attnT = sm_pool.tile([P, P], f16, tag="attnT", name="attnT")
                # ACT copy: its queue at fused start is a gen-pair shorter
                # than DVE's, so attnT lands earlier
                nc.scalar.activation(out=attnT, in_=pat, func=AF.Identity)
                return msm, attnT

            def do_smalls_b(msm, attnT):
                # AfT = Wf^T attn^T = (attn Wf)^T ; attnc = attn (bf+ba)
                pac = msm[:, P // 2 : P // 2 + 1]
                nc.tensor.matmul(pac, attnT, c_col16, start=True, stop=True)
                attnc = sm_pool.tile([P, 1], f32, tag="attnc", name="attnc")
                nc.scalar.activation(out=attnc, in_=pac, func=AF.Identity)
                paf = msm[:, P : 2 * P]
                nc.tensor.matmul(paf, wf16, attnT, start=True, stop=True)
                afT16 = sm_pool.tile([P, P], f16, tag="afT", name="afT")
                nc.vector.tensor_copy(out=afT16, in_=paf)
                paa = msm[:, 2 * P : 3 * P]
                nc.tensor.matmul(paa, wa16, attnT, start=True, stop=True)
                aaT16 = sm_pool.tile([P, P], f16, tag="aaT", name="aaT")
                nc.vector.tensor_copy(out=aaT16, in_=paa)
                return afT16, aaT16, attnc

            def do_smalls(b, attn):
                msm, attnT = do_smalls_a(b, attn)
                return do_smalls_b(msm, attnT)

            pending_stores = []

            def make_chunk_thunks(
                b, afT16, aaT16, attnc, out_tile=OUT_TILE, pools=None
            ):
                # out = AfT.T @ X + AaT.T @ Y + attnc, in OUT_CHUNK pieces.
                # Returns per-chunk closures so the driver can interleave
                # them with gen groups of the next batch.
                x_t, y_t = xy_tiles.pop(b)
                nout_ = seq // out_tile
                cpo_ = out_tile // OUT_CHUNK
                otag = "ot" if out_tile == OUT_TILE else f"ot{out_tile}"
                pools = pools or [(psum_out, "po")]
                state = {}

                def chunk(h, cc):
                    if cc == 0:
                        # 4 staging buffers: early batches hold their tiles
                        # until the deferred store flush; the final tail's
                        # store completions otherwise gate tile reuse
                        state[h] = out_pool.tile(
                            [P, out_tile], f16, tag=otag, name="ot", bufs=4
                        )
                    ot = state[h]
                    c0_ = h * out_tile + cc * OUT_CHUNK
                    cs = slice(c0_, c0_ + OUT_CHUNK)
                    pool, ptag = pools[(h * cpo_ + cc) % len(pools)]
                    po = pool.tile([P, OUT_CHUNK], f32, tag=ptag, name="po")
                    nc.tensor.matmul(po, afT16, x_t[:, cs], start=True, stop=False)
                    nc.tensor.matmul(po, aaT16, y_t[:, cs], start=False, stop=True)
                    ots = ot[:, cc * OUT_CHUNK : (cc + 1) * OUT_CHUNK]
                    # split PSUM evacuation between DVE and ACT
                    if (h * cpo_ + cc) % 2 == 0:
                        nc.vector.tensor_scalar_add(ots, po, attnc)
                    else:
                        nc.scalar.activation(
                            out=ots, in_=po, func=AF.Identity, bias=attnc
                        )
                    if cc == cpo_ - 1:
                        hs = slice(h * out_tile, (h + 1) * out_tile)
                        if b == bpc - 1:
                            # final batch: alternate both HWDGE rings (both
                            # idle by now) so two stores are in flight and
                            # the kernel-end drain starts sooner
                            eng = nc.sync if h % 2 == 0 else nc.scalar
                            eng.dma_start(out_d[b, :, hs], ot)
                        elif b == bpc - 2:
                            # loads are done; HWDGE completion (~0.6us)
                            # keeps the kernel tail short
                            nc.sync.dma_start(out_d[b, :, hs], ot)
                        else:
                            # early batches: DEFER the store until all loads
                            # are done — store traffic during the b2/b3 load
                            # window pushes HBM demand past the ~360 GB/s
                            # per-core cap and stalls the gen pipeline
                            pending_stores.append((out_d[b, :, hs], ot))

                return [
                    (lambda h=h, cc=cc: chunk(h, cc))
                    for h in range(nout_)
                    for cc in range(cpo_)
                ]

            def do_fused(bg, bt, attn, hold=0):
                # gen(bg) groups interleaved with tail(bt) out chunks AND
                # scores(bg) chunk matmuls (one gen-pair behind): the PE
                # alternates matmul bursts while ACT/DVE drain the previous
                # window's PSUM, so no phase is evacuation-gated. Holds back
                # `hold` chunks for the caller to run later.
                PRE = 3  # attn-independent gen pairs covering the softmax
                x_t, y_t = xy_tiles[bg]
                xpT = acts_pool.tile([P, seq], f16, tag="xpT", name="xpT")
                ypT = acts_pool.tile([P, seq], f16, tag="ypT", name="ypT")
                acc = ScoreAcc(bg, xpT, ypT)
                # transpose right after pair 0, dependent matmuls after
                # pair 2: the attnT copy (queued behind pair 0's x-evac on
                # ACT) completes while the PE runs pairs 1-2
                gen_group(x_t, wfT16, xpT, 0, nc.scalar)
                gen_group(y_t, waT16, ypT, 0, nc.vector)
                msm, attnT = do_smalls_a(bt, attn)
                for g in range(1, PRE):
                    gen_group(x_t, wfT16, xpT, g, nc.scalar)
                    gen_group(y_t, waT16, ypT, g, nc.vector)
                acc.run_to((PRE - 2) * GEN_GRP)
                afT16, aaT16, attnc = do_smalls_b(msm, attnT)
                thunks = make_chunk_thunks(bt, afT16, aaT16, attnc)
                ti = 0
                run = len(thunks) - hold
                for g in range(PRE, ngrp):
                    gen_group(x_t, wfT16, xpT, g, nc.scalar)
                    gen_group(y_t, waT16, ypT, g, nc.vector)
                    # two-group lag: group g-1's ACT evac (~1.15us) is not
                    # reliably done when the pair-g matmuls finish filling
                    acc.run_to((g - 1) * GEN_GRP)
                    want = (run * (g - PRE + 1)) // (ngrp - PRE)
                    while ti < want:
                        thunks[ti]()
                        ti += 1
                while ti < run:
                    thunks[ti]()
                    ti += 1
                return xpT, ypT, thunks[run:], acc.finish()

            # ---- software-pipelined driver ----
            # Per batch: softmax(b) -> fused[gen(b+1) + tail(b) +
            # scores(b+1)] -> softmax(b+1) ... The last HOLD chunks of the
            # second-to-last tail are deferred past the last softmax, and
            # the last tail runs alone with small stores on the SP ring.
            xpT, ypT, pbank = do_gen_scores(0)
            leftover = []
            for b in range(bpc):
                if b + 1 < bpc:
                    attn = do_softmax(b, pbank)
                    issue_load(b + 3)
                    if b == bpc - 2:
                        # all loads have completed by the last fused phase:
                        # flush the deferred early-batch stores on the idle
                        # SWDGE ring (their receipt lag hides behind the
                        # remaining compute)
                        for dst, ot in pending_stores:
                            nc.gpsimd.dma_start(dst, ot)
                        pending_stores.clear()
                    hold = HOLD if b == bpc - 2 else 0
                    xpT, ypT, leftover, pbank = do_fused(
                        b + 1, b, attn, hold=hold
                    )
                else:
                    # last batch: exp first; the deferred chunks cover the
                    # normalize half AND the attnT-copy latency (the smalls
                    # transpose slots in between them)
                    e, se = do_softmax_a(b, pbank)
                    for t in leftover[:2]:
                        t()
                    attn = do_softmax_b(b, e, se)
                    msm, attnT = do_smalls_a(b, attn)
                    for t in leftover[2:]:
                        t()
                    afT16, aaT16, attnc = do_smalls_b(msm, attnT)
                    # gen is done by now: alternate the chunk banks between
                    # the pout and pgen pools (4 slots) so the lone final
                    # tail is never evacuation-gated
                    for t in make_chunk_thunks(
                        b, afT16, aaT16, attnc, out_tile=1024,
                        pools=[(psum_out, "po"), (psum_gen, "pg")],
                    ):
                        t()

    _split_excess_waits(nc, mybir, max_waits=1)
    return nc


_nc_cache = {}


def _get_nc():
    key = (BPC, L)
    if key not in _nc_cache:
        _nc_cache[key] = build_nc(BPC, L)
    return _nc_cache[key]


def make_in_maps(x, y, Wf, bf, Wa, ba):
    x16 = np.asarray(x).astype(np.float16)
    y16 = np.asarray(y).astype(np.float16)
    Wf = np.asarray(Wf, dtype=np.float32)
    bf = np.asarray(bf, dtype=np.float32)
    Wa = np.asarray(Wa, dtype=np.float32)
    ba = np.asarray(ba, dtype=np.float32)

    # Bias rank-1 rows for the scores (exact, vs the fp16-quantized inputs):
    # scores = Wf G Wa^T + u ba^T + bf w^T with u = Wf (X 1), w = Wa (Y 1) + L ba
    sx = x16.astype(np.float32).sum(axis=-1)  # (B, P)
    sy = y16.astype(np.float32).sum(axis=-1)
    u = sx @ Wf.T  # (B, P)
    w = sy @ Wa.T + float(L) * ba[None, :]
    urow = u.astype(np.float16)  # (B, P)
    wrow = w.astype(np.float16)

    wpack = np.zeros((P, WP_COLS), dtype=np.float16)
    wpack[:, WP_WFT : WP_WFT + P] = Wf.T
    wpack[:, WP_WAT : WP_WAT + P] = Wa.T
    wpack[:, WP_WF : WP_WF + P] = Wf
    wpack[:, WP_WA : WP_WA + P] = Wa
    wpack[:, WP_ID : WP_ID + P] = np.eye(P, dtype=np.float16)
    wpack[:, WP_C] = (bf + ba).astype(np.float16)
    wpack = np.ascontiguousarray(wpack)

    in_maps = []
    for c in range(NCORES):
        sl = slice(c * BPC, (c + 1) * BPC)
        rpack = np.zeros((1, RP_COLS), dtype=np.float16)
        rpack[0, RP_BF : RP_BF + P] = bf.astype(np.float16)
        rpack[0, RP_BA : RP_BA + P] = ba.astype(np.float16)
        rpack[0, RP_U : RP_U + BPC * P] = urow[sl].reshape(-1)
        rpack[0, RP_W : RP_W + BPC * P] = wrow[sl].reshape(-1)
        in_maps.append(
            {
                "x": np.ascontiguousarray(x16[sl]),
                "y": np.ascontiguousarray(y16[sl]),
                "wpack": wpack,
                "rpack": rpack,
            }
        )
    return in_maps


def kernel(x, y, Wf, bf, Wa, ba):
    from concourse.bass_utils import run_bass_kernel_spmd

    in_maps = make_in_maps(x, y, Wf, bf, Wa, ba)
    nc = _get_nc()
    res = run_bass_kernel_spmd(nc, in_maps, core_ids=list(range(NCORES)))
    out = np.concatenate([r["out"] for r in res.results], axis=0)
    return np.ascontiguousarray(out.astype(np.float32))


if __name__ == "__main__":
    rng = np.random.default_rng(0)
    inputs = {
        "x": rng.standard_normal((B, P, L), dtype=np.float32),
        "y": rng.standard_normal((B, P, L), dtype=np.float32),
        "Wf": (rng.standard_normal((P, P)) / np.sqrt(P)).astype(np.float32),
        "bf": (rng.standard_normal(P) * 0.02).astype(np.float32),
        "Wa": (rng.standard_normal((P, P)) / np.sqrt(P)).astype(np.float32),
        "ba": (rng.standard_normal(P) * 0.02).astype(np.float32),
    }
    o = kernel(**inputs)
    print(o.shape, o.dtype)


# revision 55
# speedup vs baseline: 1.1761x; 1.1761x over previous
"""Biaffine attention kernel for Trainium2, data-parallel over 8 NeuronCores.

Math (per batch b, X = x[b] (128, L), Y = y[b]):
    xp = Wf X + bf 1^T,  yp = Wa Y + ba 1^T
    scores = xp yp^T
           = Wf (X Y^T) Wa^T + (Wf X 1) ba^T + bf (Wa Y 1)^T + L bf ba^T
    attn = softmax(scores, -1) / sqrt(L)
    out  = attn (xp + yp) = (attn Wf) X + (attn Wa) Y + (attn (bf+ba)) 1^T

Distribution: batch dim (32) sharded 4-per-core across 8 cores; weights
replicated. No collectives.

Implementation notes (per core):
  - x/y are cast to fp16 on the host; all HBM traffic is fp16 (in 4 MiB +
    out 2 MiB per batch vs 12 MiB for fp32), out is upcast host-side.
  - all small constants (Wf/Wa natural + transposed fp16, identity, bias
    rows, bf+ba column, score-bias rank-1 rows) are precomputed on the host
    and shipped as two packed tensors (one [128,641] + one [1,1280] DMA),
    so the kernel needs no on-chip identity build or weight transposes and
    the first gen matmul only waits on the first x/y piece.
  - a block of warm-up matmuls on a zero tile runs while the first inputs
    stream in, so the PE HAM clock-gate reaches 8/8 before the first real
    matmul (otherwise the first ~15 us run at half clock); zero-valued
    filler matmuls also pad the load-paced batch-0 ramp.
  - "gen": xpT chunks = matmul(lhsT=X_c, rhs=WfT) produce the transposed
    projections directly from the natural-layout input; PSUM groups are
    1024 cols (2 banks) so each ACT/DVE evacuation moves twice as much per
    instruction (the fixed cost per evac op is ~40% of a 512-col evac).
  - the per-batch pipeline is one fused phase: gen(b+1) groups, scores(b+1)
    chunk matmuls (one gen-pair behind the groups producing their inputs),
    and tail(b) out chunks interleave on the PE so ACT/DVE always have a
    window of evacuation work while the PE fills the next PSUM tile —
    no phase is evacuation-gated and the PE never idles long enough for
    the HAM clock-gate to re-throttle mid-kernel.
  - scores accumulate over the 64 transposed chunks into one PSUM bank;
    the three bias rank-1 terms are added exactly via two k=1 matmuls
    (u ba^T and bf (Wa sy + L ba)^T) from host-precomputed rows. The
    attn-derived smalls (attnT/AfT/AaT/attnc) live in a second psc bank
    that alternates with the scores bank, so scores(b+1) never waits on
    tail(b)'s reads.
  - out = AfT.T @ X + AaT.T @ Y consumed from the still-resident raw fp16
    inputs, in 512-col PSUM chunks, with the attn(bf+ba) column folded in
    as the PSUM-evacuation bias. The lone final tail alternates its chunk
    banks across the pout AND (by then idle) pgen pools and its stores
    across both HWDGE rings, with 4 staging buffers.
  - stores: SWDGE ring for early batches (gpsimd is otherwise idle), SP
    ring for the last two (loads done; HWDGE receipt keeps the tail short).
  - the second-to-last tail's last HOLD chunks are deferred past the last
    softmax so the PE stays busy through its serial chain.
"""

import numpy as np

P = 128
L = 8192
B = 32
NCORES = 8
BPC = B // NCORES  # batches per core
SQRT_L = float(np.sqrt(float(L)))

GEN_GRP = 8  # gen chunks per PSUM evacuation (8*128 fp32 = 2 banks)
OUT_TILE = 4096  # out staging tile (fp16) per DMA store
OUT_CHUNK = 512  # out matmul PSUM chunk (1 bank)
IN_PIECE = 4096  # input DMA piece (cols; 2 MiB transfers run nearer peak BW)
NWARM = 50  # PE warm-up matmuls (N=128 each, ~5 us at the cold clock)
HOLD = 5  # tail chunks deferred past the last softmax

# wpack column layout (fp16, [128, 641])
WP_WFT = 0  # Wf^T  (f x o)
WP_WAT = 128  # Wa^T  (a x o)
WP_WF = 256  # Wf    (o x f)
WP_WA = 384  # Wa    (o x a)
WP_ID = 512  # identity
WP_C = 640  # bf+ba column
WP_COLS = 641
# rpack column layout (fp16, [1, 256 + 2*BPC*128])
RP_BF = 0
RP_BA = 128
RP_U = 256
RP_W = 256 + BPC * P
RP_COLS = 256 + 2 * BPC * P


def _patch_tail_drain(tile, mybir, ScopedClock):
    """This container's walrus rejects >1 sync wait on the kernel-tail Drain
    (setupSyncWait: 'Too many sync wait commands'). Spread the tail-drain
    waits across a chain of drains, one wait each."""
    if getattr(tile.TileContext, "_drain_split_patched", False):
        return

    def _split_drain_and_barrier(self, tick_clock, wait_clock):
        nc = self.nc
        drain_inst = nc.sync.drain()
        wait_clock.add_sem_waits(
            drain_inst.ins, ScopedClock({None: tick_clock.global_clock})
        )
        si = drain_inst.ins.sync_info
        if si is not None and si.on_wait is not None and len(si.on_wait) > 1:
            waits = list(si.on_wait)
            si.on_wait = waits[:1]
            # spread the remaining waits across all engines as parallel
            # single-wait NOP chains (walrus allows only one wait per
            # instruction; a serial chain on SP costs ~4us of kernel tail)
            engines = [nc.tensor, nc.vector, nc.scalar, nc.gpsimd, nc.sync]
            for i, w in enumerate(waits[1:]):
                extra = engines[i % len(engines)].nop(nofuse=True)
                esi = extra.ins.sync_info
                if esi is None:
                    extra.ins.sync_info = mybir.SyncInfo(on_wait=[w], on_update=[])
                else:
                    ow = list(esi.on_wait) if esi.on_wait else []
                    ow.append(w)
                    esi.on_wait = ow
        nc.all_engine_barrier(sem_only=True)
        assert self.sems is not None
        popped = nc._tile_sem_poison_stack.pop()
        assert popped is self._sem_poison
        nc.clear_and_free_semaphores(list(self.sems.allocated().values()))
        nc.all_engine_barrier(sem_only=True)

    tile.TileContext._drain_and_barrier = _split_drain_and_barrier
    tile.TileContext._drain_split_patched = True


def _split_excess_waits(nc, mybir, max_waits=1):
    """Walrus in this container rejects instructions carrying more than a
    couple of sync waits ('Too many sync wait commands'). Hoist excess waits
    onto dedicated same-engine NoOps inserted just before the instruction."""
    ctr = 0
    for blk in nc.m.functions[0].blocks:
        new_insts = []
        for inst in blk.instructions:
            si = inst.sync_info
            if si is not None and si.on_wait and len(si.on_wait) > max_waits:
                waits = list(si.on_wait)
                excess, keep = waits[:-max_waits], waits[-max_waits:]
                si.on_wait = keep
                for i in range(0, len(excess), max_waits):
                    ctr += 1
                    nop = mybir.InstNoOp(
                        name=f"I-waitsplit-{ctr}",
                        sync_info=mybir.SyncInfo(
                            on_wait=excess[i : i + max_waits], on_update=[]
                        ),
                        bass_nofuse=True,
                        engine=inst.engine,
                    )
                    nc.register_instruction(nop)
                    new_insts.append(nop)
            new_insts.append(inst)
        blk.instructions = new_insts


def build_nc(bpc=BPC, seq=L):
    import concourse.bass as bass
    import concourse.mybir as mybir
    import concourse.tile as tile
    from concourse.vector_clock import ScopedClock

    _patch_tail_drain(tile, mybir, ScopedClock)

    f32 = mybir.dt.float32
    f16 = mybir.dt.float16
    AF = mybir.ActivationFunctionType
    ALU = mybir.AluOpType
    AX = mybir.AxisListType

    sqrt_l = float(np.sqrt(float(seq)))
    ntr = seq // P  # number of 128-col chunks
    ngrp = ntr // GEN_GRP
    npc = seq // IN_PIECE  # input dma pieces

    nc = bass.Bass("TRN2", target_bir_lowering=False, debug=False)
    x_d = nc.dram_tensor("x", [bpc, P, seq], f16, kind="ExternalInput").ap()
    y_d = nc.dram_tensor("y", [bpc, P, seq], f16, kind="ExternalInput").ap()
    wpack_d = nc.dram_tensor("wpack", [P, WP_COLS], f16, kind="ExternalInput").ap()
    rpack_d = nc.dram_tensor("rpack", [1, RP_COLS], f16, kind="ExternalInput").ap()
    out_d = nc.dram_tensor("out", [bpc, P, seq], f16, kind="ExternalOutput").ap()

    def load_inputs(b, x_t, y_t):
        # b0 uses graduated pieces with x/y interleaved so the gen pairs
        # (which consume x then y of the same column range) unblock in
        # arrival order. Everything rides the SP ring: the ACT ring measured
        # ~100 GB/s when both rings contend for the SDMA engines.
        if b == 0:
            edges = [0, 1024, 2048, 4096, seq]
        elif b == 1:
            # gen(1) inside fused(1,0) tracks these arrivals closely:
            # finer pieces unblock its first pairs sooner
            edges = [0, 2048, 4096, 6144, seq]
        else:
            # every batch's load completes only just before its gen phase
            # consumes the last pieces — keep piece-granular arrivals
            edges = [p_ * IN_PIECE for p_ in range(npc + 1)]
        for e0, e1 in zip(edges[:-1], edges[1:]):
            ps_ = slice(e0, e1)
            nc.sync.dma_start(x_t[:, ps_], x_d[b, :, ps_])
            nc.sync.dma_start(y_t[:, ps_], y_d[b, :, ps_])

    with tile.TileContext(nc) as tc:
        with (
            tc.tile_pool(name="consts", bufs=1) as consts,
            tc.tile_pool(name="xin", bufs=3) as xin_pool,
            tc.tile_pool(name="acts", bufs=2) as acts_pool,
            tc.tile_pool(name="sm", bufs=2) as sm_pool,
            tc.tile_pool(name="outs", bufs=2) as out_pool,
            tc.tile_pool(name="pgen", bufs=2, space="PSUM") as psum_gen,
            tc.tile_pool(name="pout", bufs=2, space="PSUM") as psum_out,
            tc.tile_pool(name="psc", bufs=2, space="PSUM") as psum_sc,
        ):
            # ---- PE warm-up: dummy matmuls on a zero tile while the first
            # inputs stream in. Keeps the PE HAM busy-window fed so the
            # clock gate opens (4/8 -> 8/8) before the first real matmul.
            # The warm bank comes from the psc pool; its slot is free again
            # immediately (nothing reads it). ----
            warm_ps = psum_sc.tile([P, 4 * P], f32, tag="ps", name="warm_ps")
            wtile = consts.tile([P, P], f16)
            nc.vector.memset(wtile, 0.0)
            for _ in range(NWARM):
                nc.tensor.matmul(warm_ps[:, 0:P], wtile, wtile, start=True, stop=True)

            # ---- constants: one packed DMA each on the ACT ring ----
            wpk = consts.tile([P, WP_COLS], f16)
            nc.scalar.dma_start(wpk, wpack_d)
            rpk = consts.tile([1, RP_COLS], f16)
            nc.scalar.dma_start(rpk, rpack_d)
            wfT16 = wpk[:, WP_WFT : WP_WFT + P]
            waT16 = wpk[:, WP_WAT : WP_WAT + P]
            wf16 = wpk[:, WP_WF : WP_WF + P]
            wa16 = wpk[:, WP_WA : WP_WA + P]
            ids16 = wpk[:, WP_ID : WP_ID + P]
            c_col16 = wpk[:, WP_C : WP_C + 1]
            bf_row16 = rpk[:, RP_BF : RP_BF + P]
            ba_row16 = rpk[:, RP_BA : RP_BA + P]

            xy_tiles = {}

            def issue_load(b):
                if b >= bpc:
                    return
                x_t = xin_pool.tile([P, seq], f16, tag="x_t", name="x_t")
                y_t = xin_pool.tile([P, seq], f16, tag="y_t", name="y_t")
                load_inputs(b, x_t, y_t)
                xy_tiles[b] = (x_t, y_t)

            issue_load(0)
            for b0 in range(1, min(3, bpc)):
                issue_load(b0)

            def gen_group(src, wT, dst, g, eng):
                # one gen group: GEN_GRP transposed-projection matmuls into a
                # 2-bank PSUM tile, then a single 1024-col evacuation
                pg = psum_gen.tile([P, GEN_GRP * P], f32, tag="pg", name="pg")
                for t in range(GEN_GRP):
                    c = g * GEN_GRP + t
                    cs = slice(c * P, (c + 1) * P)
                    ts_ = slice(t * P, (t + 1) * P)
                    nc.tensor.matmul(
                        pg[:, ts_], src[:, cs], wT, start=True, stop=True
                    )
                gs = slice(g * GEN_GRP * P, (g + 1) * GEN_GRP * P)
                if eng is nc.scalar:
                    nc.scalar.activation(out=dst[:, gs], in_=pg, func=AF.Identity)
                else:
                    nc.vector.tensor_copy(out=dst[:, gs], in_=pg)

            class ScoreAcc:
                """scores(b) = sum_c xpT_c^T ypT_c + rank-1 bias terms,
                accumulated into one PSUM bank. The chunk matmuls are issued
                incrementally (one gen-pair behind the gen groups producing
                xpT/ypT) so they fill PE time while ACT/DVE drain PSUM."""

                def __init__(self, b, xpT, ypT):
                    self.b = b
                    self.xpT, self.ypT = xpT, ypT
                    self.pbank = psum_sc.tile(
                        [P, 4 * P], f32, tag="ps", name="pbank"
                    )
                    self.done = 0

                def run_to(self, c_hi):
                    ps = self.pbank[:, 0:P]
                    for c in range(self.done, c_hi):
                        cs = slice(c * P, (c + 1) * P)
                        nc.tensor.matmul(
                            ps, self.xpT[:, cs], self.ypT[:, cs],
                            start=(c == 0), stop=False,
                        )
                    self.done = max(self.done, c_hi)

                def finish(self):
                    self.run_to(ntr)
                    ps = self.pbank[:, 0:P]
                    us = slice(RP_U + self.b * P, RP_U + (self.b + 1) * P)
                    ws = slice(RP_W + self.b * P, RP_W + (self.b + 1) * P)
                    nc.tensor.matmul(
                        ps, rpk[:, us], ba_row16, start=False, stop=False
                    )
                    nc.tensor.matmul(
                        ps, bf_row16, rpk[:, ws], start=False, stop=True
                    )
                    return self.pbank

            def do_gen_scores(b):
                # batch-0 bootstrap: gen pairs paced by the input stream,
                # with scores chunks trailing one pair behind. Zero-valued
                # filler matmuls (accumulating 0 into unused scores-bank
                # columns) keep the PE HAM busy-fraction up while pairs wait
                # on the DMA stream, so the clock gate stays at 8/8.
                x_t, y_t = xy_tiles[b]
                xpT = acts_pool.tile([P, seq], f16, tag="xpT", name="xpT")
                ypT = acts_pool.tile([P, seq], f16, tag="ypT", name="ypT")
                acc = ScoreAcc(b, xpT, ypT)
                for g in range(ngrp):
                    gen_group(x_t, wfT16, xpT, g, nc.scalar)
                    gen_group(y_t, waT16, ypT, g, nc.vector)
                    acc.run_to(g * GEN_GRP)
                    for _ in range(8):
                        nc.tensor.matmul(
                            acc.pbank[:, 3 * P : 4 * P], wtile, wtile,
                            start=False, stop=False,
                        )
                return xpT, ypT, acc.finish()

            def do_softmax_a(b, pbank):
                # softmax part 1: max-reduce + exp (DVE + ACT)
                ps = pbank[:, 0:P]
                negmx = sm_pool.tile([P, 1], f32, tag="negmx", name="negmx")
                nc.vector.tensor_reduce(
                    out=negmx, in_=ps, axis=AX.X, op=ALU.max, negate=True
                )
                e = sm_pool.tile([P, P], f32, tag="e", name="e")
                se = sm_pool.tile([P, 1], f32, tag="se", name="se")
                nc.scalar.activation(
                    out=e, in_=ps, func=AF.Exp, bias=negmx, scale=1.0, accum_out=se
                )
                return e, se

            def do_softmax_b(b, e, se):
                # softmax part 2: normalize (DVE only)
                sse = sm_pool.tile([P, 1], f32, tag="sse", name="sse")
                nc.vector.tensor_scalar_mul(sse, se, sqrt_l)
                rcp = sm_pool.tile([P, 1], f32, tag="rcp", name="rcp")
                nc.vector.reciprocal(rcp, sse)
                attn = sm_pool.tile([P, P], f16, tag="attn", name="attn")
                nc.vector.tensor_scalar_mul(attn, e, rcp)
                return attn

            def do_softmax(b, pbank):
                e, se = do_softmax_a(b, pbank)
                return do_softmax_b(b, e, se)

            def do_smalls_a(b, attn):
                # attn transpose, in its own psc bank (so the scores bank
                # frees as soon as exp has read it)
                msm = psum_sc.tile([P, 4 * P], f32, tag="ps", name="msm")
                pat = msm[:, 0 : P // 2].bitcast(f16)
                nc.tensor.transpose(pat, attn, ids16)
                attnT = sm_pool.tile([P, P], f16, tag="attnT", name="attnT")
                # ACT copy: its queue at fused start is a gen-pair shorter
                # than DVE's, so attnT lands earlier
                nc.scalar.activation(out=attnT, in_=pat, func=AF.Identity)
                return msm, attnT

            def do_smalls_b(msm, attnT):
                # AfT = Wf^T attn^T = (attn Wf)^T ; attnc = attn (bf+ba)
                pac = msm[:, P // 2 : P // 2 + 1]
                nc.tensor.matmul(pac, attnT, c_col16, start=True, stop=True)
                attnc = sm_pool.tile([P, 1], f32, tag="attnc", name="attnc")
                nc.scalar.activation(out=attnc, in_=pac, func=AF.Identity)
                paf = msm[:, P : 2 * P]
                nc.tensor.matmul(paf, wf16, attnT, start=True, stop=True)
                afT16 = sm_pool.tile([P, P], f16, tag="afT", name="afT")
                nc.vector.tensor_copy(out=afT16, in_=paf)
                paa = msm[:, 2 * P : 3 * P]
                nc.tensor.matmul(paa, wa16, attnT, start=True, stop=True)
                aaT16 = sm_pool.tile([P, P], f16, tag="aaT", name="aaT")
                nc.vector.tensor_copy(out=aaT16, in_=paa)
                return afT16, aaT16, attnc

            def do_smalls(b, attn):
                msm, attnT = do_smalls_a(b, attn)
                return do_smalls_b(msm, attnT)

            pending_stores = []

            def make_chunk_thunks(
                b, afT16, aaT16, attnc, out_tile=OUT_TILE, pools=None
            ):
                # out = AfT.T @ X + AaT.T @ Y + attnc, in OUT_CHUNK pieces.
                # Returns per-chunk closures so the driver can interleave
                # them with gen groups of the next batch.
                x_t, y_t = xy_tiles.pop(b)
                nout_ = seq // out_tile
                cpo_ = out_tile // OUT_CHUNK
                otag = "ot" if out_tile == OUT_TILE else f"ot{out_tile}"
                pools = pools or [(psum_out, "po")]
                state = {}

                def chunk(h, cc):
                    if cc == 0:
                        # 4 staging buffers: early batches hold their tiles
                        # until the deferred store flush; the final tail's
                        # store completions otherwise gate tile reuse
                        state[h] = out_pool.tile(
                            [P, out_tile], f16, tag=otag, name="ot", bufs=4
                        )
                    ot = state[h]
                    c0_ = h * out_tile + cc * OUT_CHUNK
                    cs = slice(c0_, c0_ + OUT_CHUNK)
                    pool, ptag = pools[(h * cpo_ + cc) % len(pools)]
                    po = pool.tile([P, OUT_CHUNK], f32, tag=ptag, name="po")
                    nc.tensor.matmul(po, afT16, x_t[:, cs], start=True, stop=False)
                    nc.tensor.matmul(po, aaT16, y_t[:, cs], start=False, stop=True)
                    ots = ot[:, cc * OUT_CHUNK : (cc + 1) * OUT_CHUNK]
                    # split PSUM evacuation between DVE and ACT
                    if (h * cpo_ + cc) % 2 == 0:
                        nc.vector.tensor_scalar_add(ots, po, attnc)
                    else:
                        nc.scalar.activation(
                            out=ots, in_=po, func=AF.Identity, bias=attnc
                        )
                    if cc == cpo_ - 1:
                        hs = slice(h * out_tile, (h + 1) * out_tile)
                        if b == bpc - 1:
                            # final batch: alternate both HWDGE rings (both
                            # idle by now) so two stores are in flight and
                            # the kernel-end drain starts sooner
                            eng = nc.sync if h % 2 == 0 else nc.scalar
                            eng.dma_start(out_d[b, :, hs], ot)
                        elif b == bpc - 2:
                            # loads are done; HWDGE completion (~0.6us)
                            # keeps the kernel tail short
                            nc.sync.dma_start(out_d[b, :, hs], ot)
                        else:
                            # early batches: DEFER the store until all loads
                            # are done — store traffic during the b2/b3 load
                            # window pushes HBM demand past the ~360 GB/s
                            # per-core cap and stalls the gen pipeline
                            pending_stores.append((out_d[b, :, hs], ot))

                return [
                    (lambda h=h, cc=cc: chunk(h, cc))
                    for h in range(nout_)
                    for cc in range(cpo_)
                ]

            def do_fused(bg, bt, attn, hold=0):
                # gen(bg) groups interleaved with tail(bt) out chunks AND
                # scores(bg) chunk matmuls (one gen-pair behind): the PE
                # alternates matmul bursts while ACT/DVE drain the previous
                # window's PSUM, so no phase is evacuation-gated. Holds back
                # `hold` chunks for the caller to run later.
                PRE = 3  # attn-independent gen pairs covering the softmax
                x_t, y_t = xy_tiles[bg]
                xpT = acts_pool.tile([P, seq], f16, tag="xpT", name="xpT")
                ypT = acts_pool.tile([P, seq], f16, tag="ypT", name="ypT")
                acc = ScoreAcc(bg, xpT, ypT)
                # transpose right after pair 0, dependent matmuls after
                # pair 2: the attnT copy (queued behind pair 0's x-evac on
                # ACT) completes while the PE runs pairs 1-2
                gen_group(x_t, wfT16, xpT, 0, nc.scalar)
                gen_group(y_t, waT16, ypT, 0, nc.vector)
                msm, attnT = do_smalls_a(bt, attn)
                for g in range(1, PRE):
                    gen_group(x_t, wfT16, xpT, g, nc.scalar)
                    gen_group(y_t, waT16, ypT, g, nc.vector)
                acc.run_to((PRE - 2) * GEN_GRP)
                afT16, aaT16, attnc = do_smalls_b(msm, attnT)
                thunks = make_chunk_thunks(bt, afT16, aaT16, attnc)
                ti = 0
                run = len(thunks) - hold
                for g in range(PRE, ngrp):
                    gen_group(x_t, wfT16, xpT, g, nc.scalar)
                    gen_group(y_t, waT16, ypT, g, nc.vector)
                    # two-group lag: group g-1's ACT evac (~1.15us) is not
                    # reliably done when the pair-g matmuls finish filling
                    acc.run_to((g - 1) * GEN_GRP)
                    want = (run * (g - PRE + 1)) // (ngrp - PRE)
                    while ti < want:
                        thunks[ti]()
                        ti += 1
                while ti < run:
                    thunks[ti]()
                    ti += 1
                return xpT, ypT, thunks[run:], acc.finish()

            # ---- software-pipelined driver ----
            # Per batch: softmax(b) -> fused[gen(b+1) + tail(b) +
            # scores(b+1)] -> softmax(b+1) ... The last HOLD chunks of the
            # second-to-last tail are deferred past the last softmax, and
            # the last tail runs alone with small stores on the SP ring.
            xpT, ypT, pbank = do_gen_scores(0)
            leftover = []
            for b in range(bpc):
                if b + 1 < bpc:
                    attn = do_softmax(b, pbank)
                    issue_load(b + 3)
                    if b == bpc - 2:
                        # all loads have completed by the last fused phase:
                        # flush the deferred early-batch stores on the idle
                        # SWDGE ring (their receipt lag hides behind the
                        # remaining compute)
                        for dst, ot in pending_stores:
                            nc.gpsimd.dma_start(dst, ot)
                        pending_stores.clear()
                    hold = HOLD if b == bpc - 2 else 0
                    xpT, ypT, leftover, pbank = do_fused(
                        b + 1, b, attn, hold=hold
                    )
                else:
                    # last batch: exp first; the deferred chunks cover the
                    # normalize half AND the attnT-copy latency (the smalls
                    # transpose slots in between them)
                    e, se = do_softmax_a(b, pbank)
                    for t in leftover[:2]:
                        t()
                    attn = do_softmax_b(b, e, se)
                    msm, attnT = do_smalls_a(b, attn)
                    for t in leftover[2:]:
                        t()
                    afT16, aaT16, attnc = do_smalls_b(msm, attnT)
                    # gen is done by now: alternate the chunk banks between
                    # the pout and pgen pools (4 slots) so the lone final
                    # tail is never evacuation-gated
                    for t in make_chunk_thunks(
                        b, afT16, aaT16, attnc, out_tile=1024,
                        pools=[(psum_out, "po"), (psum_gen, "pg")],
                    ):
                        t()

    _split_excess_waits(nc, mybir, max_waits=1)
    return nc


_nc_cache = {}


def _get_nc():
    key = (BPC, L)
    if key not in _nc_cache:
        _nc_cache[key] = build_nc(BPC, L)
    return _nc_cache[key]


def make_in_maps(x, y, Wf, bf, Wa, ba):
    x16 = np.asarray(x).astype(np.float16)
    y16 = np.asarray(y).astype(np.float16)
    Wf = np.asarray(Wf, dtype=np.float32)
    bf = np.asarray(bf, dtype=np.float32)
    Wa = np.asarray(Wa, dtype=np.float32)
    ba = np.asarray(ba, dtype=np.float32)

    # Bias rank-1 rows for the scores (exact, vs the fp16-quantized inputs):
    # scores = Wf G Wa^T + u ba^T + bf w^T with u = Wf (X 1), w = Wa (Y 1) + L ba
    sx = x16.astype(np.float32).sum(axis=-1)  # (B, P)
    sy = y16.astype(np.float32).sum(axis=-1)
    u = sx @ Wf.T  # (B, P)
    w = sy @ Wa.T + float(L) * ba[None, :]
    urow = u.astype(np.float16)  # (B, P)
    wrow = w.astype(np.float16)

    wpack = np.zeros((P, WP_COLS), dtype=np.float16)
    wpack[:, WP_WFT : WP_WFT + P] = Wf.T
    wpack[:, WP_WAT : WP_WAT + P] = Wa.T
    wpack[:, WP_WF : WP_WF + P] = Wf
    wpack[:, WP_WA : WP_WA + P] = Wa
    wpack[:, WP_ID : WP_ID + P] = np.eye(P, dtype=np.float16)
    wpack[:, WP_C] = (bf + ba).astype(np.float16)
    wpack = np.ascontiguousarray(wpack)

    in_maps = []
    for c in range(NCORES):
        sl = slice(c * BPC, (c + 1) * BPC)
        rpack = np.zeros((1, RP_COLS), dtype=np.float16)
        rpack[0, RP_BF : RP_BF + P] = bf.astype(np.float16)
        rpack[0, RP_BA : RP_BA + P] = ba.astype(np.float16)
        rpack[0, RP_U : RP_U + BPC * P] = urow[sl].reshape(-1)
        rpack[0, RP_W : RP_W + BPC * P] = wrow[sl].reshape(-1)
        in_maps.append(
            {
                "x": np.ascontiguousarray(x16[sl]),
                "y": np.ascontiguousarray(y16[sl]),
                "wpack": wpack,
                "rpack": rpack,
            }
        )
    return in_maps


def kernel(x, y, Wf, bf, Wa, ba):
    from concourse.bass_utils import run_bass_kernel_spmd

    in_maps = make_in_maps(x, y, Wf, bf, Wa, ba)
    nc = _get_nc()
    res = run_bass_kernel_spmd(nc, in_maps, core_ids=list(range(NCORES)))
    out = np.concatenate([r["out"] for r in res.results], axis=0)
    return np.ascontiguousarray(out.astype(np.float32))


if __name__ == "__main__":
    rng = np.random.default_rng(0)
    inputs = {
        "x": rng.standard_normal((B, P, L), dtype=np.float32),
        "y": rng.standard_normal((B, P, L), dtype=np.float32),
        "Wf": (rng.standard_normal((P, P)) / np.sqrt(P)).astype(np.float32),
        "bf": (rng.standard_normal(P) * 0.02).astype(np.float32),
        "Wa": (rng.standard_normal((P, P)) / np.sqrt(P)).astype(np.float32),
        "ba": (rng.standard_normal(P) * 0.02).astype(np.float32),
    }
    o = kernel(**inputs)
    print(o.shape, o.dtype)
